# revision 16
# baseline (speedup 1.0000x reference)
"""Trainium2 kernel for nn_CanCountLeaveOperator: pairwise triu ops on [1,4096] inputs.

For every pair (i0, i1) with i0 <= i1 (row-major triu order, M = n(n+1)/2):
  leaves = x_leaves[i0] + x_leaves[i1] + 1            (int32)
  cplx   = x_cplx[i0] + x_cplx[i1] + 1                (float32)
  prime  = (a%b==0 or b%a==0) ? max(a,b) : 0          (int32), a=x_prime[i0], b=x_prime[i1]

Sharding: rows i (= i0) are dealt to 8 cores round-robin at 128-row block
granularity: core k owns row blocks b with b % 8 == k (b = i // 128). Each core
runs an IDENTICAL program over 4 row blocks s=0..3 with padded widths
W_s = 4096 - 1024*s; the per-core row shift (128*k) is folded into the host-side
input prep (x shifted by 128*k), so one SPMD program serves all 8 cores.

Divisibility on device (no int div/mod on the DVE): host supplies 1/x (f32
scalars per row, fp16 per column); f1 = rne_i16(b * (1/a)) via the engine
output-dtype conversion (HW rounds to nearest); then b%a==0 <=> f1*a == b.
fp16 reciprocals are safe: |a*fl16(1/b) - a/b| <= (a/b)*2^-11 < 0.5 for
quotients < 1024, so the rounded quotient is exact whenever divisibility holds,
and the verify-multiply rejects everything else (products that could collide
with a compare target <= 999 are exactly representable).
out = max(b*[b%a==0], a*[a%b==0]) which equals max(a,b) iff either divides.

Engine assignment follows measured op costs (ns per 2048 elems):
ts 16-bit 4x=664 (chained too), tt f16 2x=1132-1226, stt always 1x=2262,
ACT=2000, GPSIMD tt=~6000. DVE gets ts-heavy work, ACT the two activation
streams, GPSIMD one off-critical-path tensor_tensor.
"""

import sys

sys.path.insert(0, "/opt/trn_rl_repo")

import numpy as np

N = 4096
P = 128
NCORES = 8
SPB = 4  # row blocks per core
WS = [4096, 3072, 2048, 1024]  # padded block widths
OFFS = [0, 4096, 7168, 9216]  # column offset of each block in the output
TOTW = 10240
CH = 2048  # max compute chunk width (columns)
LOADW = 2048  # input piece width for load/compute overlap
M = N * (N + 1) // 2

_compiled = None


def _build():
    import concourse.bacc as bacc
    import concourse.mybir as mybir
    from concourse.tile import TileContext

    AF = mybir.ActivationFunctionType
    OP = mybir.AluOpType
    dt = mybir.dt

    nc = bacc.Bacc(None, debug=False)

    xl = nc.dram_tensor("xl", [P, N], dt.int8, kind="ExternalInput")
    xc = nc.dram_tensor("xc", [P, N], dt.float32, kind="ExternalInput")
    xp16 = nc.dram_tensor("xp16", [P, N], dt.float16, kind="ExternalInput")
    xr16 = nc.dram_tensor("xr16", [P, N], dt.float16, kind="ExternalInput")
    sl = nc.dram_tensor("sl", [P, SPB], dt.float32, kind="ExternalInput")
    sc = nc.dram_tensor("sc", [P, SPB], dt.float32, kind="ExternalInput")
    sa = nc.dram_tensor("sa", [P, SPB], dt.float32, kind="ExternalInput")
    sr = nc.dram_tensor("sr", [P, SPB], dt.float32, kind="ExternalInput")

    ol = nc.dram_tensor("ol", [P, TOTW], dt.int8, kind="ExternalOutput")
    oc = nc.dram_tensor("oc", [P, TOTW], dt.float32, kind="ExternalOutput")
    op_ = nc.dram_tensor("op", [P, TOTW], dt.float16, kind="ExternalOutput")

    with TileContext(nc) as tc:
        with (
            tc.tile_pool(name="xin", bufs=1) as xin,
            tc.tile_pool(name="scal", bufs=1) as scal,
            tc.tile_pool(name="work", bufs=3) as work,
            tc.tile_pool(name="outp", bufs=2) as outp,
        ):
            xl_t = xin.tile([P, N], dt.int8, tag="xl")
            xc_t = xin.tile([P, N], dt.float32, tag="xc")
            xp_t = xin.tile([P, N], dt.float16, tag="xp")
            xr_t = xin.tile([P, N], dt.float16, tag="xr")
            # load in pieces so compute can start before the full row is in
            for u in range(N // LOADW):
                c0, c1 = u * LOADW, (u + 1) * LOADW
                nc.sync.dma_start(xl_t[:, c0:c1], xl[:, c0:c1])
                nc.sync.dma_start(xc_t[:, c0:c1], xc[:, c0:c1])
                nc.sync.dma_start(xp_t[:, c0:c1], xp16[:, c0:c1])
                nc.sync.dma_start(xr_t[:, c0:c1], xr16[:, c0:c1])

            sl_t = scal.tile([P, SPB], dt.float32, tag="sl")
            sc_t = scal.tile([P, SPB], dt.float32, tag="sc")
            sa_t = scal.tile([P, SPB], dt.float32, tag="sa")
            sr_t = scal.tile([P, SPB], dt.float32, tag="sr")
            nc.sync.dma_start(sl_t[:], sl[:])
            nc.sync.dma_start(sc_t[:], sc[:])
            nc.sync.dma_start(sa_t[:], sa[:])
            nc.sync.dma_start(sr_t[:], sr[:])

            for s in range(SPB):
                sl_s = sl_t[:, s : s + 1]
                sc_s = sc_t[:, s : s + 1]
                sa_s = sa_t[:, s : s + 1]
                sr_s = sr_t[:, s : s + 1]

                w_s = WS[s]
                ol_b = outp.tile([P, w_s], dt.int8, tag="ol")
                oc_b = outp.tile([P, w_s], dt.float32, tag="oc")
                op_b = outp.tile([P, w_s], dt.float16, tag="op")

                q0 = 0
                while q0 < w_s:
                    cw = min(CH, w_s - q0)
                    q1 = q0 + cw
                    c0 = 1024 * s + q0  # column in shifted-x coords
                    c1 = c0 + cw
                    xp_sl = xp_t[:, c0:c1]

                    # f1 = rne_i16(b / a)            [ACT, issued first: feeds DVE]
                    f1 = work.tile([P, CH], dt.int16, tag="f1")
                    nc.scalar.activation(f1[:, :cw], xp_sl, AF.Copy, scale=sr_s)
                    # f2 = rne_i16(a / b)            [ACT]
                    f2 = work.tile([P, CH], dt.int16, tag="f2")
                    nc.scalar.activation(f2[:, :cw], xr_t[:, c0:c1], AF.Copy, scale=sa_s)

                    # leaves = x[i1] + (x[i0] + 1)   [ACT]
                    nc.scalar.activation(
                        ol_b[:, q0:q1], xl_t[:, c0:c1], AF.Identity, bias=sl_s
                    )
                    # cplx = x[i1] + (x[i0] + 1)     [ACT]
                    nc.scalar.activation(
                        oc_b[:, q0:q1], xc_t[:, c0:c1], AF.Identity, bias=sc_s
                    )

                    # z1 = (f1*a == b)               [DVE stt 1x]
                    z1 = work.tile([P, CH], dt.float16, tag="z1")
                    nc.vector.scalar_tensor_tensor(
                        z1[:, :cw], f1[:, :cw], sa_s, xp_sl, OP.mult, OP.is_equal
                    )
                    # w = b*f2 (f16 overflow saturates, always != a then) [DVE stt 1x]
                    w = work.tile([P, CH], dt.float16, tag="w")
                    nc.vector.scalar_tensor_tensor(
                        w[:, :cw], xp_sl, 1.0, f2[:, :cw], OP.mult, OP.mult
                    )
                    # z2a = (w == a) * a             [DVE chained ts 4x]
                    z2a = work.tile([P, CH], dt.float16, tag="z2a")
                    nc.vector.tensor_scalar(
                        z2a[:, :cw], w[:, :cw], sa_s, sa_s, OP.is_equal, OP.mult
                    )
                    # z1b = z1 * b                   [DVE tt f16 2x]
                    z1b = work.tile([P, CH], dt.float16, tag="z1b")
                    nc.vector.tensor_tensor(z1b[:, :cw], z1[:, :cw], xp_sl, OP.mult)
                    # out = max(z1b, z2a)            [DVE tt f16 2x]
                    nc.vector.tensor_tensor(
                        op_b[:, q0:q1], z1b[:, :cw], z2a[:, :cw], OP.max
                    )
                    q0 = q1

                o0, o1 = OFFS[s], OFFS[s] + w_s
                nc.gpsimd.dma_start(ol[:, o0:o1], ol_b[:])
                nc.sync.dma_start(oc[:, o0:o1], oc_b[:])
                nc.gpsimd.dma_start(op_[:, o0:o1], op_b[:])

    nc.compile()
    return nc


def _get_compiled():
    global _compiled
    if _compiled is None:
        _compiled = _build()
    return _compiled


def _prep_core(k, lv, cv, pv):
    """Build the per-core input map. lv/cv/pv are the [4096] host vectors."""
    pad = 128 * k
    lk = np.concatenate([lv[pad:], np.ones(pad, lv.dtype)])
    ck = np.concatenate([cv[pad:], np.ones(pad, cv.dtype)])
    pk = np.concatenate([pv[pad:], np.ones(pad, pv.dtype)])

    pkf = pk.astype(np.float32)
    rk = (1.0 / pkf).astype(np.float32)

    ridx = 1024 * np.arange(SPB)[None, :] + np.arange(P)[:, None]  # [P, SPB]

    return {
        "xl": np.ascontiguousarray(np.broadcast_to(lk.astype(np.int8), (P, N))),
        "xc": np.ascontiguousarray(np.broadcast_to(ck.astype(np.float32), (P, N))),
        "xp16": np.ascontiguousarray(np.broadcast_to(pkf.astype(np.float16), (P, N))),
        "xr16": np.ascontiguousarray(np.broadcast_to(rk.astype(np.float16), (P, N))),
        "sl": (lk[ridx] + 1).astype(np.float32),
        "sc": (ck[ridx] + 1.0).astype(np.float32),
        "sa": pkf[ridx].copy(),
        "sr": rk[ridx].copy(),
    }


def kernel(x_leaves, x_cplx, x_prime):
    from concourse.bass_utils import run_bass_kernel_spmd

    nc = _get_compiled()

    lv = np.asarray(x_leaves).reshape(-1).astype(np.int32)
    cv = np.asarray(x_cplx).reshape(-1).astype(np.float32)
    pv = np.asarray(x_prime).reshape(-1).astype(np.int32)

    in_maps = [_prep_core(k, lv, cv, pv) for k in range(NCORES)]
    res = run_bass_kernel_spmd(nc, in_maps, list(range(NCORES))).results

    leaves = np.empty((1, M), np.int32)
    cplx = np.empty((1, M), np.float32)
    prime = np.empty((1, M), np.int32)

    off = 0
    for b in range(N // P):
        k = b % NCORES
        s = b // NCORES
        r = res[k]
        olk, ock, opk = r["ol"], r["oc"], r["op"]
        for p in range(P):
            i = P * b + p
            L = N - i
            a0 = OFFS[s] + p
            leaves[0, off : off + L] = olk[p, a0 : a0 + L]
            cplx[0, off : off + L] = ock[p, a0 : a0 + L]
            prime[0, off : off + L] = opk[p, a0 : a0 + L]
            off += L
    assert off == M
    return leaves, cplx, prime


# revision 17
# speedup vs baseline: 1.1342x; 1.1342x over previous
"""Trainium2 kernel for nn_CanCountLeaveOperator: pairwise triu ops on [1,4096] inputs.

For every pair (i0, i1) with i0 <= i1 (row-major triu order, M = n(n+1)/2):
  leaves = x_leaves[i0] + x_leaves[i1] + 1            (int32)
  cplx   = x_cplx[i0] + x_cplx[i1] + 1                (float32)
  prime  = (a%b==0 or b%a==0) ? max(a,b) : 0          (int32), a=x_prime[i0], b=x_prime[i1]

Sharding: rows i (= i0) are dealt to 8 cores round-robin at 128-row block
granularity: core k owns row blocks b with b % 8 == k (b = i // 128). Each core
runs an IDENTICAL program over 4 row blocks s=0..3 with padded widths
W_s = 4096 - 1024*s; the per-core row shift (128*k) is folded into the host-side
input prep (x shifted by 128*k), so one SPMD program serves all 8 cores.

Divisibility on device (no int div/mod on the DVE): host supplies 1/x (f32
scalars per row, fp16 per column); f1 = rne_i16(b * (1/a)) via the engine
output-dtype conversion (HW rounds to nearest); then b%a==0 <=> f1*a == b.
fp16 reciprocals are safe: |a*fl16(1/b) - a/b| <= (a/b)*2^-11 < 0.5 for
quotients < 1024, so the rounded quotient is exact whenever divisibility holds,
and the verify-multiply rejects everything else (products that could collide
with a compare target <= 999 are exactly representable).
out = max(b*[b%a==0], a*[a%b==0]) which equals max(a,b) iff either divides.

Engine assignment follows measured op costs (ns per 2048 elems):
ts 16-bit 4x=664 (chained too), tt f16 2x=1132-1226, stt always 1x=2262,
ACT=2000, GPSIMD tt=~6000. DVE gets ts-heavy work, ACT the two activation
streams, GPSIMD one off-critical-path tensor_tensor.
"""

import sys

sys.path.insert(0, "/opt/trn_rl_repo")

import numpy as np

N = 4096
P = 128
NCORES = 8
SPB = 4  # row blocks per core
WS = [4096, 3072, 2048, 1024]  # padded block widths
OFFS = [0, 4096, 7168, 9216]  # column offset of each block in the output
TOTW = 10240
CH = 2048  # max compute chunk width (columns)
LOADW = 2048  # input piece width for load/compute overlap
M = N * (N + 1) // 2

_compiled = None


def _build():
    import concourse.bacc as bacc
    import concourse.mybir as mybir
    from concourse.tile import TileContext

    AF = mybir.ActivationFunctionType
    OP = mybir.AluOpType
    dt = mybir.dt

    nc = bacc.Bacc(None, debug=False)

    xl = nc.dram_tensor("xl", [P, N], dt.int8, kind="ExternalInput")
    xc = nc.dram_tensor("xc", [P, N], dt.float32, kind="ExternalInput")
    xp16 = nc.dram_tensor("xp16", [P, N], dt.float16, kind="ExternalInput")
    xr16 = nc.dram_tensor("xr16", [P, N], dt.float16, kind="ExternalInput")
    sl = nc.dram_tensor("sl", [P, SPB], dt.float32, kind="ExternalInput")
    sc = nc.dram_tensor("sc", [P, SPB], dt.float32, kind="ExternalInput")
    sa = nc.dram_tensor("sa", [P, SPB], dt.float32, kind="ExternalInput")
    sr = nc.dram_tensor("sr", [P, SPB], dt.float32, kind="ExternalInput")

    ol = nc.dram_tensor("ol", [P, TOTW], dt.int8, kind="ExternalOutput")
    oc = nc.dram_tensor("oc", [P, TOTW], dt.float32, kind="ExternalOutput")
    op_ = nc.dram_tensor("op", [P, TOTW], dt.float16, kind="ExternalOutput")

    with TileContext(nc) as tc:
        with (
            tc.tile_pool(name="xin", bufs=1) as xin,
            tc.tile_pool(name="scal", bufs=1) as scal,
            tc.tile_pool(name="work", bufs=3) as work,
            tc.tile_pool(name="outp", bufs=2) as outp,
        ):
            xl_t = xin.tile([P, N], dt.int8, tag="xl")
            xc_t = xin.tile([P, N], dt.float32, tag="xc")
            xp_t = xin.tile([P, N], dt.float16, tag="xp")
            xr_t = xin.tile([P, N], dt.float16, tag="xr")
            # load in pieces so compute can start before the full row is in
            for u in range(N // LOADW):
                c0, c1 = u * LOADW, (u + 1) * LOADW
                nc.sync.dma_start(xl_t[:, c0:c1], xl[:, c0:c1])
                nc.sync.dma_start(xc_t[:, c0:c1], xc[:, c0:c1])
                nc.sync.dma_start(xp_t[:, c0:c1], xp16[:, c0:c1])
                nc.sync.dma_start(xr_t[:, c0:c1], xr16[:, c0:c1])

            sl_t = scal.tile([P, SPB], dt.float32, tag="sl")
            sc_t = scal.tile([P, SPB], dt.float32, tag="sc")
            sa_t = scal.tile([P, SPB], dt.float32, tag="sa")
            sr_t = scal.tile([P, SPB], dt.float32, tag="sr")
            nc.sync.dma_start(sl_t[:], sl[:])
            nc.sync.dma_start(sc_t[:], sc[:])
            nc.sync.dma_start(sa_t[:], sa[:])
            nc.sync.dma_start(sr_t[:], sr[:])

            for s in range(SPB):
                sl_s = sl_t[:, s : s + 1]
                sc_s = sc_t[:, s : s + 1]
                sa_s = sa_t[:, s : s + 1]
                sr_s = sr_t[:, s : s + 1]

                w_s = WS[s]
                ol_b = outp.tile([P, w_s], dt.int8, tag="ol")
                oc_b = outp.tile([P, w_s], dt.float32, tag="oc")
                op_b = outp.tile([P, w_s], dt.float16, tag="op")

                q0 = 0
                while q0 < w_s:
                    cw = min(CH, w_s - q0)
                    q1 = q0 + cw
                    c0 = 1024 * s + q0  # column in shifted-x coords
                    c1 = c0 + cw
                    xp_sl = xp_t[:, c0:c1]

                    # f1 = rne_i16(b / a)            [ACT, issued first: feeds DVE]
                    f1 = work.tile([P, CH], dt.int16, tag="f1")
                    nc.scalar.activation(f1[:, :cw], xp_sl, AF.Copy, scale=sr_s)
                    # f2 = rne_i16(a / b)            [DVE ts 4x]
                    f2 = work.tile([P, CH], dt.int16, tag="f2")
                    nc.vector.tensor_scalar(f2[:, :cw], xr_t[:, c0:c1], sa_s, None, OP.mult)

                    # leaves = x[i1] + (x[i0] + 1)   [ACT]
                    nc.scalar.activation(
                        ol_b[:, q0:q1], xl_t[:, c0:c1], AF.Identity, bias=sl_s
                    )
                    # cplx = x[i1] + (x[i0] + 1)     [ACT]
                    nc.scalar.activation(
                        oc_b[:, q0:q1], xc_t[:, c0:c1], AF.Identity, bias=sc_s
                    )

                    # z1 = (f1*a == b)               [DVE stt 1x]
                    z1 = work.tile([P, CH], dt.float16, tag="z1")
                    nc.vector.scalar_tensor_tensor(
                        z1[:, :cw], f1[:, :cw], sa_s, xp_sl, OP.mult, OP.is_equal
                    )
                    # w = b*f2 (f16 overflow saturates, always != a then) [DVE stt 1x]
                    w = work.tile([P, CH], dt.float16, tag="w")
                    nc.vector.scalar_tensor_tensor(
                        w[:, :cw], xp_sl, 1.0, f2[:, :cw], OP.mult, OP.mult
                    )
                    # z2a = (w == a) * a             [DVE chained ts 4x]
                    z2a = work.tile([P, CH], dt.float16, tag="z2a")
                    nc.vector.tensor_scalar(
                        z2a[:, :cw], w[:, :cw], sa_s, sa_s, OP.is_equal, OP.mult
                    )
                    # z1b = z1 * b                   [DVE tt f16 2x]
                    z1b = work.tile([P, CH], dt.float16, tag="z1b")
                    nc.vector.tensor_tensor(z1b[:, :cw], z1[:, :cw], xp_sl, OP.mult)
                    # out = max(z1b, z2a)            [DVE tt f16 2x]
                    nc.vector.tensor_tensor(
                        op_b[:, q0:q1], z1b[:, :cw], z2a[:, :cw], OP.max
                    )
                    q0 = q1

                o0, o1 = OFFS[s], OFFS[s] + w_s
                nc.gpsimd.dma_start(ol[:, o0:o1], ol_b[:])
                nc.sync.dma_start(oc[:, o0:o1], oc_b[:])
                nc.gpsimd.dma_start(op_[:, o0:o1], op_b[:])

    nc.compile()
    return nc


def _get_compiled():
    global _compiled
    if _compiled is None:
        _compiled = _build()
    return _compiled


def _prep_core(k, lv, cv, pv):
    """Build the per-core input map. lv/cv/pv are the [4096] host vectors."""
    pad = 128 * k
    lk = np.concatenate([lv[pad:], np.ones(pad, lv.dtype)])
    ck = np.concatenate([cv[pad:], np.ones(pad, cv.dtype)])
    pk = np.concatenate([pv[pad:], np.ones(pad, pv.dtype)])

    pkf = pk.astype(np.float32)
    rk = (1.0 / pkf).astype(np.float32)

    ridx = 1024 * np.arange(SPB)[None, :] + np.arange(P)[:, None]  # [P, SPB]

    return {
        "xl": np.ascontiguousarray(np.broadcast_to(lk.astype(np.int8), (P, N))),
        "xc": np.ascontiguousarray(np.broadcast_to(ck.astype(np.float32), (P, N))),
        "xp16": np.ascontiguousarray(np.broadcast_to(pkf.astype(np.float16), (P, N))),
        "xr16": np.ascontiguousarray(np.broadcast_to(rk.astype(np.float16), (P, N))),
        "sl": (lk[ridx] + 1).astype(np.float32),
        "sc": (ck[ridx] + 1.0).astype(np.float32),
        "sa": pkf[ridx].copy(),
        "sr": rk[ridx].copy(),
    }


def kernel(x_leaves, x_cplx, x_prime):
    from concourse.bass_utils import run_bass_kernel_spmd

    nc = _get_compiled()

    lv = np.asarray(x_leaves).reshape(-1).astype(np.int32)
    cv = np.asarray(x_cplx).reshape(-1).astype(np.float32)
    pv = np.asarray(x_prime).reshape(-1).astype(np.int32)

    in_maps = [_prep_core(k, lv, cv, pv) for k in range(NCORES)]
    res = run_bass_kernel_spmd(nc, in_maps, list(range(NCORES))).results

    leaves = np.empty((1, M), np.int32)
    cplx = np.empty((1, M), np.float32)
    prime = np.empty((1, M), np.int32)

    off = 0
    for b in range(N // P):
        k = b % NCORES
        s = b // NCORES
        r = res[k]
        olk, ock, opk = r["ol"], r["oc"], r["op"]
        for p in range(P):
            i = P * b + p
            L = N - i
            a0 = OFFS[s] + p
            leaves[0, off : off + L] = olk[p, a0 : a0 + L]
            cplx[0, off : off + L] = ock[p, a0 : a0 + L]
            prime[0, off : off + L] = opk[p, a0 : a0 + L]
            off += L
    assert off == M
    return leaves, cplx, prime


# revision 18
# speedup vs baseline: 1.1912x; 1.0502x over previous
"""Trainium2 kernel for nn_CanCountLeaveOperator: pairwise triu ops on [1,4096] inputs.

For every pair (i0, i1) with i0 <= i1 (row-major triu order, M = n(n+1)/2):
  leaves = x_leaves[i0] + x_leaves[i1] + 1            (int32)
  cplx   = x_cplx[i0] + x_cplx[i1] + 1                (float32)
  prime  = (a%b==0 or b%a==0) ? max(a,b) : 0          (int32), a=x_prime[i0], b=x_prime[i1]

Sharding: rows i (= i0) are dealt to 8 cores round-robin at 128-row block
granularity: core k owns row blocks b with b % 8 == k (b = i // 128). Each core
runs an IDENTICAL program over 4 row blocks s=0..3 with padded widths
W_s = 4096 - 1024*s; the per-core row shift (128*k) is folded into the host-side
input prep (x shifted by 128*k), so one SPMD program serves all 8 cores.

Divisibility on device (no int div/mod on the DVE): host supplies 1/x (f32
scalars per row, fp16 per column); f1 = rne_i16(b * (1/a)) via the engine
output-dtype conversion (HW rounds to nearest); then b%a==0 <=> f1*a == b.
fp16 reciprocals are safe: |a*fl16(1/b) - a/b| <= (a/b)*2^-11 < 0.5 for
quotients < 1024, so the rounded quotient is exact whenever divisibility holds,
and the verify-multiply rejects everything else (products that could collide
with a compare target <= 999 are exactly representable).
out = max(b*[b%a==0], a*[a%b==0]) which equals max(a,b) iff either divides.

Engine assignment follows measured op costs (ns per 2048 elems):
ts 16-bit 4x=664 (chained too), tt f16 2x=1132-1226, stt always 1x=2262,
ACT=2000, GPSIMD tt=~6000. DVE gets ts-heavy work, ACT the two activation
streams, GPSIMD one off-critical-path tensor_tensor.
"""

import sys

sys.path.insert(0, "/opt/trn_rl_repo")

import numpy as np

N = 4096
P = 128
NCORES = 8
SPB = 4  # row blocks per core
WS = [4096, 3072, 2048, 1024]  # padded block widths
OFFS = [0, 4096, 7168, 9216]  # column offset of each block in the output
TOTW = 10240
CH = 2048  # max compute chunk width (columns)
LOADW = 2048  # input piece width for load/compute overlap
M = N * (N + 1) // 2

_compiled = None


def _build():
    import concourse.bacc as bacc
    import concourse.mybir as mybir
    from concourse.tile import TileContext

    AF = mybir.ActivationFunctionType
    OP = mybir.AluOpType
    dt = mybir.dt

    nc = bacc.Bacc(None, debug=False)

    xl = nc.dram_tensor("xl", [P, N], dt.int8, kind="ExternalInput")
    xc = nc.dram_tensor("xc", [P, N], dt.float32, kind="ExternalInput")
    xp16 = nc.dram_tensor("xp16", [P, N], dt.float16, kind="ExternalInput")
    xr16 = nc.dram_tensor("xr16", [P, N], dt.float16, kind="ExternalInput")
    sl = nc.dram_tensor("sl", [P, SPB], dt.float32, kind="ExternalInput")
    sc = nc.dram_tensor("sc", [P, SPB], dt.float32, kind="ExternalInput")
    sa = nc.dram_tensor("sa", [P, SPB], dt.float32, kind="ExternalInput")
    sr = nc.dram_tensor("sr", [P, SPB], dt.float32, kind="ExternalInput")

    ol = nc.dram_tensor("ol", [P, TOTW], dt.int8, kind="ExternalOutput")
    oc = nc.dram_tensor("oc", [P, TOTW], dt.float32, kind="ExternalOutput")
    op_ = nc.dram_tensor("op", [P, TOTW], dt.float16, kind="ExternalOutput")

    with TileContext(nc) as tc:
        with (
            tc.tile_pool(name="xin", bufs=1) as xin,
            tc.tile_pool(name="scal", bufs=1) as scal,
            tc.tile_pool(name="work", bufs=3) as work,
            tc.tile_pool(name="outp", bufs=2) as outp,
        ):
            xl_t = xin.tile([P, N], dt.int8, tag="xl")
            xc_t = xin.tile([P, N], dt.float32, tag="xc")
            xp_t = xin.tile([P, N], dt.float16, tag="xp")
            xr_t = xin.tile([P, N], dt.float16, tag="xr")
            # load in pieces so compute can start before the full row is in
            for u in range(N // LOADW):
                c0, c1 = u * LOADW, (u + 1) * LOADW
                nc.sync.dma_start(xl_t[:, c0:c1], xl[:, c0:c1])
                nc.sync.dma_start(xc_t[:, c0:c1], xc[:, c0:c1])
                nc.sync.dma_start(xp_t[:, c0:c1], xp16[:, c0:c1])
                nc.sync.dma_start(xr_t[:, c0:c1], xr16[:, c0:c1])

            sl_t = scal.tile([P, SPB], dt.float32, tag="sl")
            sc_t = scal.tile([P, SPB], dt.float32, tag="sc")
            sa_t = scal.tile([P, SPB], dt.float32, tag="sa")
            sr_t = scal.tile([P, SPB], dt.float32, tag="sr")
            nc.sync.dma_start(sl_t[:], sl[:])
            nc.sync.dma_start(sc_t[:], sc[:])
            nc.sync.dma_start(sa_t[:], sa[:])
            nc.sync.dma_start(sr_t[:], sr[:])

            for s in range(SPB):
                sl_s = sl_t[:, s : s + 1]
                sc_s = sc_t[:, s : s + 1]
                sa_s = sa_t[:, s : s + 1]
                sr_s = sr_t[:, s : s + 1]

                w_s = WS[s]
                ol_b = outp.tile([P, w_s], dt.int8, tag="ol")
                oc_b = outp.tile([P, w_s], dt.float32, tag="oc")
                op_b = outp.tile([P, w_s], dt.float16, tag="op")

                q0 = 0
                while q0 < w_s:
                    cw = min(CH, w_s - q0)
                    q1 = q0 + cw
                    c0 = 1024 * s + q0  # column in shifted-x coords
                    c1 = c0 + cw
                    xp_sl = xp_t[:, c0:c1]

                    # f1 = rne_i16(b / a)            [ACT, issued first: feeds DVE]
                    f1 = work.tile([P, CH], dt.int16, tag="f1")
                    nc.scalar.activation(f1[:, :cw], xp_sl, AF.Copy, scale=sr_s)
                    # f2 = rne_i16(a / b)            [DVE ts 4x]
                    f2 = work.tile([P, CH], dt.int16, tag="f2")
                    nc.vector.tensor_scalar(f2[:, :cw], xr_t[:, c0:c1], sa_s, None, OP.mult)

                    # leaves = x[i1] + (x[i0] + 1)   [ACT]
                    nc.scalar.activation(
                        ol_b[:, q0:q1], xl_t[:, c0:c1], AF.Identity, bias=sl_s
                    )
                    # cplx = x[i1] + (x[i0] + 1)     [ACT]
                    nc.scalar.activation(
                        oc_b[:, q0:q1], xc_t[:, c0:c1], AF.Identity, bias=sc_s
                    )

                    # g1 = f1*a (<= b + a/2 + eps <= 1499: exact in fp16) [DVE ts 4x]
                    g1 = work.tile([P, CH], dt.float16, tag="g1")
                    nc.vector.tensor_scalar(g1[:, :cw], f1[:, :cw], sa_s, None, OP.mult)
                    # z1 = (g1 == b)                 [DVE tt f16 2x]
                    z1 = work.tile([P, CH], dt.float16, tag="z1")
                    nc.vector.tensor_tensor(z1[:, :cw], g1[:, :cw], xp_sl, OP.is_equal)
                    # w = b*f2 (f16 overflow saturates, always != a then) [DVE stt 1x]
                    w = work.tile([P, CH], dt.float16, tag="w")
                    nc.vector.scalar_tensor_tensor(
                        w[:, :cw], xp_sl, 1.0, f2[:, :cw], OP.mult, OP.mult
                    )
                    # z2a = (w == a) * a             [DVE chained ts 4x]
                    z2a = work.tile([P, CH], dt.float16, tag="z2a")
                    nc.vector.tensor_scalar(
                        z2a[:, :cw], w[:, :cw], sa_s, sa_s, OP.is_equal, OP.mult
                    )
                    # z1b = z1 * b                   [DVE tt f16 2x]
                    z1b = work.tile([P, CH], dt.float16, tag="z1b")
                    nc.vector.tensor_tensor(z1b[:, :cw], z1[:, :cw], xp_sl, OP.mult)
                    # out = max(z1b, z2a)            [DVE tt f16 2x]
                    nc.vector.tensor_tensor(
                        op_b[:, q0:q1], z1b[:, :cw], z2a[:, :cw], OP.max
                    )
                    q0 = q1

                o0, o1 = OFFS[s], OFFS[s] + w_s
                nc.gpsimd.dma_start(ol[:, o0:o1], ol_b[:])
                nc.sync.dma_start(oc[:, o0:o1], oc_b[:])
                nc.gpsimd.dma_start(op_[:, o0:o1], op_b[:])

    nc.compile()
    return nc


def _get_compiled():
    global _compiled
    if _compiled is None:
        _compiled = _build()
    return _compiled


def _prep_core(k, lv, cv, pv):
    """Build the per-core input map. lv/cv/pv are the [4096] host vectors."""
    pad = 128 * k
    lk = np.concatenate([lv[pad:], np.ones(pad, lv.dtype)])
    ck = np.concatenate([cv[pad:], np.ones(pad, cv.dtype)])
    pk = np.concatenate([pv[pad:], np.ones(pad, pv.dtype)])

    pkf = pk.astype(np.float32)
    rk = (1.0 / pkf).astype(np.float32)

    ridx = 1024 * np.arange(SPB)[None, :] + np.arange(P)[:, None]  # [P, SPB]

    return {
        "xl": np.ascontiguousarray(np.broadcast_to(lk.astype(np.int8), (P, N))),
        "xc": np.ascontiguousarray(np.broadcast_to(ck.astype(np.float32), (P, N))),
        "xp16": np.ascontiguousarray(np.broadcast_to(pkf.astype(np.float16), (P, N))),
        "xr16": np.ascontiguousarray(np.broadcast_to(rk.astype(np.float16), (P, N))),
        "sl": (lk[ridx] + 1).astype(np.float32),
        "sc": (ck[ridx] + 1.0).astype(np.float32),
        "sa": pkf[ridx].copy(),
        "sr": rk[ridx].copy(),
    }


def kernel(x_leaves, x_cplx, x_prime):
    from concourse.bass_utils import run_bass_kernel_spmd

    nc = _get_compiled()

    lv = np.asarray(x_leaves).reshape(-1).astype(np.int32)
    cv = np.asarray(x_cplx).reshape(-1).astype(np.float32)
    pv = np.asarray(x_prime).reshape(-1).astype(np.int32)

    in_maps = [_prep_core(k, lv, cv, pv) for k in range(NCORES)]
    res = run_bass_kernel_spmd(nc, in_maps, list(range(NCORES))).results

    leaves = np.empty((1, M), np.int32)
    cplx = np.empty((1, M), np.float32)
    prime = np.empty((1, M), np.int32)

    off = 0
    for b in range(N // P):
        k = b % NCORES
        s = b // NCORES
        r = res[k]
        olk, ock, opk = r["ol"], r["oc"], r["op"]
        for p in range(P):
            i = P * b + p
            L = N - i
            a0 = OFFS[s] + p
            leaves[0, off : off + L] = olk[p, a0 : a0 + L]
            cplx[0, off : off + L] = ock[p, a0 : a0 + L]
            prime[0, off : off + L] = opk[p, a0 : a0 + L]
            off += L
    assert off == M
    return leaves, cplx, prime


# revision 19
# speedup vs baseline: 1.2790x; 1.0738x over previous
"""Trainium2 kernel for nn_CanCountLeaveOperator: pairwise triu ops on [1,4096] inputs.

For every pair (i0, i1) with i0 <= i1 (row-major triu order, M = n(n+1)/2):
  leaves = x_leaves[i0] + x_leaves[i1] + 1            (int32)
  cplx   = x_cplx[i0] + x_cplx[i1] + 1                (float32)
  prime  = (a%b==0 or b%a==0) ? max(a,b) : 0          (int32), a=x_prime[i0], b=x_prime[i1]

Sharding: rows i (= i0) are dealt to 8 cores round-robin at 128-row block
granularity: core k owns row blocks b with b % 8 == k (b = i // 128). Each core
runs an IDENTICAL program over 4 row blocks s=0..3 with padded widths
W_s = 4096 - 1024*s; the per-core row shift (128*k) is folded into the host-side
input prep (x shifted by 128*k), so one SPMD program serves all 8 cores.

Divisibility on device (no int div/mod on the DVE): host supplies 1/x (f32
scalars per row, fp16 per column); f1 = rne_i16(b * (1/a)) via the engine
output-dtype conversion (HW rounds to nearest); then b%a==0 <=> f1*a == b.
fp16 reciprocals are safe: |a*fl16(1/b) - a/b| <= (a/b)*2^-11 < 0.5 for
quotients < 1024, so the rounded quotient is exact whenever divisibility holds,
and the verify-multiply rejects everything else (products that could collide
with a compare target <= 999 are exactly representable).
out = max(b*[b%a==0], a*[a%b==0]) which equals max(a,b) iff either divides.

Engine assignment follows measured op costs (ns per 2048 elems):
ts 16-bit 4x=664 (chained too), tt f16 2x=1132-1226, stt always 1x=2262,
ACT=2000, GPSIMD tt=~6000. DVE gets ts-heavy work, ACT the two activation
streams, GPSIMD one off-critical-path tensor_tensor.
"""

import sys

sys.path.insert(0, "/opt/trn_rl_repo")

import numpy as np

N = 4096
P = 128
NCORES = 8
SPB = 4  # row blocks per core
WS = [4096, 3072, 2048, 1024]  # padded block widths
OFFS = [0, 4096, 7168, 9216]  # column offset of each block in the output
TOTW = 10240
CH = 2048  # max compute chunk width (columns)
LOADW = 2048  # input piece width for load/compute overlap
M = N * (N + 1) // 2

_compiled = None


def _build():
    import concourse.bacc as bacc
    import concourse.mybir as mybir
    from concourse.tile import TileContext

    AF = mybir.ActivationFunctionType
    OP = mybir.AluOpType
    dt = mybir.dt

    nc = bacc.Bacc(None, debug=False)

    xl = nc.dram_tensor("xl", [P, N], dt.int8, kind="ExternalInput")
    xc = nc.dram_tensor("xc", [P, N], dt.float32, kind="ExternalInput")
    xp16 = nc.dram_tensor("xp16", [P, N], dt.float16, kind="ExternalInput")
    xr16 = nc.dram_tensor("xr16", [P, N], dt.float16, kind="ExternalInput")
    sl = nc.dram_tensor("sl", [P, SPB], dt.float32, kind="ExternalInput")
    sc = nc.dram_tensor("sc", [P, SPB], dt.float32, kind="ExternalInput")
    sa = nc.dram_tensor("sa", [P, SPB], dt.float32, kind="ExternalInput")
    sr = nc.dram_tensor("sr", [P, SPB], dt.float32, kind="ExternalInput")

    ol = nc.dram_tensor("ol", [P, TOTW], dt.int8, kind="ExternalOutput")
    oc = nc.dram_tensor("oc", [P, TOTW], dt.float32, kind="ExternalOutput")
    op_ = nc.dram_tensor("op", [P, TOTW], dt.float16, kind="ExternalOutput")

    with TileContext(nc) as tc:
        with (
            tc.tile_pool(name="xin", bufs=1) as xin,
            tc.tile_pool(name="scal", bufs=1) as scal,
            tc.tile_pool(name="work", bufs=3) as work,
            tc.tile_pool(name="outp", bufs=2) as outp,
        ):
            xl_t = xin.tile([P, N], dt.int8, tag="xl")
            xc_t = xin.tile([P, N], dt.float32, tag="xc")
            xp_t = xin.tile([P, N], dt.float16, tag="xp")
            xr_t = xin.tile([P, N], dt.float16, tag="xr")
            # load in pieces so compute can start before the full row is in
            for u in range(N // LOADW):
                c0, c1 = u * LOADW, (u + 1) * LOADW
                nc.sync.dma_start(xl_t[:, c0:c1], xl[:, c0:c1])
                nc.sync.dma_start(xc_t[:, c0:c1], xc[:, c0:c1])
                nc.sync.dma_start(xp_t[:, c0:c1], xp16[:, c0:c1])
                nc.sync.dma_start(xr_t[:, c0:c1], xr16[:, c0:c1])

            sl_t = scal.tile([P, SPB], dt.float32, tag="sl")
            sc_t = scal.tile([P, SPB], dt.float32, tag="sc")
            sa_t = scal.tile([P, SPB], dt.float32, tag="sa")
            sr_t = scal.tile([P, SPB], dt.float32, tag="sr")
            nc.sync.dma_start(sl_t[:], sl[:])
            nc.sync.dma_start(sc_t[:], sc[:])
            nc.sync.dma_start(sa_t[:], sa[:])
            nc.sync.dma_start(sr_t[:], sr[:])

            for s in range(SPB):
                sl_s = sl_t[:, s : s + 1]
                sc_s = sc_t[:, s : s + 1]
                sa_s = sa_t[:, s : s + 1]
                sr_s = sr_t[:, s : s + 1]

                w_s = WS[s]
                ol_b = outp.tile([P, w_s], dt.int8, tag="ol")
                oc_b = outp.tile([P, w_s], dt.float32, tag="oc")
                op_b = outp.tile([P, w_s], dt.float16, tag="op")

                q0 = 0
                while q0 < w_s:
                    cw = min(CH, w_s - q0)
                    q1 = q0 + cw
                    c0 = 1024 * s + q0  # column in shifted-x coords
                    c1 = c0 + cw
                    xp_sl = xp_t[:, c0:c1]

                    # f1 = rne_i16(b / a)            [ACT, issued first: feeds DVE]
                    f1 = work.tile([P, CH], dt.int16, tag="f1")
                    nc.scalar.activation(f1[:, :cw], xp_sl, AF.Copy, scale=sr_s)
                    # f2 = rne_i16(a / b)            [DVE ts 4x]
                    f2 = work.tile([P, CH], dt.int16, tag="f2")
                    nc.vector.tensor_scalar(f2[:, :cw], xr_t[:, c0:c1], sa_s, None, OP.mult)

                    # leaves = x[i1] + (x[i0] + 1)   [ACT]
                    nc.scalar.activation(
                        ol_b[:, q0:q1], xl_t[:, c0:c1], AF.Identity, bias=sl_s
                    )
                    # cplx = x[i1] + (x[i0] + 1)     [ACT]
                    nc.scalar.activation(
                        oc_b[:, q0:q1], xc_t[:, c0:c1], AF.Identity, bias=sc_s
                    )

                    # g1 = f1*a (<= b + a/2 + eps <= 1499: exact in fp16) [DVE ts 4x]
                    g1 = work.tile([P, CH], dt.float16, tag="g1")
                    nc.vector.tensor_scalar(g1[:, :cw], f1[:, :cw], sa_s, None, OP.mult)
                    # z1 = (g1 == b)                 [DVE tt f16 2x]
                    z1 = work.tile([P, CH], dt.float16, tag="z1")
                    nc.vector.tensor_tensor(z1[:, :cw], g1[:, :cw], xp_sl, OP.is_equal)
                    # w = b*f2 (<= a + b/2 <= 1499: exact in fp16)  [DVE tt]
                    w = work.tile([P, CH], dt.float16, tag="w")
                    nc.vector.tensor_tensor(w[:, :cw], xp_sl, f2[:, :cw], OP.mult)
                    # z2a = (w == a) * a             [DVE chained ts 4x]
                    z2a = work.tile([P, CH], dt.float16, tag="z2a")
                    nc.vector.tensor_scalar(
                        z2a[:, :cw], w[:, :cw], sa_s, sa_s, OP.is_equal, OP.mult
                    )
                    # z1b = z1 * b                   [DVE tt f16 2x]
                    z1b = work.tile([P, CH], dt.float16, tag="z1b")
                    nc.vector.tensor_tensor(z1b[:, :cw], z1[:, :cw], xp_sl, OP.mult)
                    # out = max(z1b, z2a)            [DVE tt f16 2x]
                    nc.vector.tensor_tensor(
                        op_b[:, q0:q1], z1b[:, :cw], z2a[:, :cw], OP.max
                    )
                    q0 = q1

                o0, o1 = OFFS[s], OFFS[s] + w_s
                nc.gpsimd.dma_start(ol[:, o0:o1], ol_b[:])
                nc.sync.dma_start(oc[:, o0:o1], oc_b[:])
                nc.gpsimd.dma_start(op_[:, o0:o1], op_b[:])

    nc.compile()
    return nc


def _get_compiled():
    global _compiled
    if _compiled is None:
        _compiled = _build()
    return _compiled


def _prep_core(k, lv, cv, pv):
    """Build the per-core input map. lv/cv/pv are the [4096] host vectors."""
    pad = 128 * k
    lk = np.concatenate([lv[pad:], np.ones(pad, lv.dtype)])
    ck = np.concatenate([cv[pad:], np.ones(pad, cv.dtype)])
    pk = np.concatenate([pv[pad:], np.ones(pad, pv.dtype)])

    pkf = pk.astype(np.float32)
    rk = (1.0 / pkf).astype(np.float32)

    ridx = 1024 * np.arange(SPB)[None, :] + np.arange(P)[:, None]  # [P, SPB]

    return {
        "xl": np.ascontiguousarray(np.broadcast_to(lk.astype(np.int8), (P, N))),
        "xc": np.ascontiguousarray(np.broadcast_to(ck.astype(np.float32), (P, N))),
        "xp16": np.ascontiguousarray(np.broadcast_to(pkf.astype(np.float16), (P, N))),
        "xr16": np.ascontiguousarray(np.broadcast_to(rk.astype(np.float16), (P, N))),
        "sl": (lk[ridx] + 1).astype(np.float32),
        "sc": (ck[ridx] + 1.0).astype(np.float32),
        "sa": pkf[ridx].copy(),
        "sr": rk[ridx].copy(),
    }


def kernel(x_leaves, x_cplx, x_prime):
    from concourse.bass_utils import run_bass_kernel_spmd

    nc = _get_compiled()

    lv = np.asarray(x_leaves).reshape(-1).astype(np.int32)
    cv = np.asarray(x_cplx).reshape(-1).astype(np.float32)
    pv = np.asarray(x_prime).reshape(-1).astype(np.int32)

    in_maps = [_prep_core(k, lv, cv, pv) for k in range(NCORES)]
    res = run_bass_kernel_spmd(nc, in_maps, list(range(NCORES))).results

    leaves = np.empty((1, M), np.int32)
    cplx = np.empty((1, M), np.float32)
    prime = np.empty((1, M), np.int32)

    off = 0
    for b in range(N // P):
        k = b % NCORES
        s = b // NCORES
        r = res[k]
        olk, ock, opk = r["ol"], r["oc"], r["op"]
        for p in range(P):
            i = P * b + p
            L = N - i
            a0 = OFFS[s] + p
            leaves[0, off : off + L] = olk[p, a0 : a0 + L]
            cplx[0, off : off + L] = ock[p, a0 : a0 + L]
            prime[0, off : off + L] = opk[p, a0 : a0 + L]
            off += L
    assert off == M
    return leaves, cplx, prime
